# revision 23
# baseline (speedup 1.0000x reference)
"""AttendAndSpell (LAS decoder) Trainium2 Bass kernel, v4.

Data-parallel over batch (B=64 -> 8 items/core on 8 cores), no collectives.

v3 -> v4 (software pipelining + engine rebalance):
  - Step loop split into open/close matmul halves: each LSTM layer's psum
    groups open with the bias/embedding MM plus the *previous*-state
    operands (ready early) and close with the just-computed operand, so PE
    streams through the softmax window and the act/cell tails instead of
    idling.  L0(t+1) opens inside attention(t); L1 opens during L0's tail.
  - emb/bias matmuls in fp8 DoubleRow (were fp16): half the PE stream time.
  - GpSimd carries attention group 1's scale/copy chain + history copies
    (was 100% idle); softmax max-subtract dropped (|e| < 0.1 measured,
    f32 exp exact) and the sum fused into the Exp via accum_out.
  - s1/context history written straight into a resident SBUF slab (no DRAM
    roundtrip); output projection [128,1024]@[1024,512] chunks interleaved
    into the loop's PE-idle tails (one chunk per 2 steps), remainder in
    phase 3.
"""

import math

import numpy as np
import ml_dtypes

import concourse.bacc as bacc
import concourse.mybir as mybir
import concourse.tile as tile
from concourse.bass_utils import run_bass_kernel_spmd
from concourse.masks import make_identity

B, R, T, H, V = 64, 256, 128, 512, 4096
NCORES = 8
BS = B // NCORES  # 8
G = 4 * H  # 2048
KC = 2 * H  # 1024
KT_H = H // 128  # 4
KT_KC = KC // 128  # 8
RT = R // 128  # 2
S = 512.0  # fp8 weight scale
F32 = mybir.dt.float32
F16 = mybir.dt.float16
E4 = mybir.dt.float8e4
AF = mybir.ActivationFunctionType
ALU = mybir.AluOpType
AX = mybir.AxisListType
DR = mybir.MatmulPerfMode.DoubleRow

# chunk -> gate: 0=i 1=f 2=o 3=g; process g,i,f,o
CH_ORDER = (3, 0, 1, 2)


def build_program(T_steps=T):
    nc = bacc.Bacc(None, target_bir_lowering=False)

    d_hT = nc.dram_tensor("hT", [H, BS * R], F16, kind="ExternalInput")
    d_W0 = nc.dram_tensor("W0", [128, 4, 2, G], E4, kind="ExternalInput")
    d_W1 = nc.dram_tensor("W1", [128, 4, 2, G], E4, kind="ExternalInput")
    d_phiT = nc.dram_tensor("phiT", [H, H], F16, kind="ExternalInput")
    d_psiT = nc.dram_tensor("psiT", [H, H], F16, kind="ExternalInput")
    d_psib_bc = nc.dram_tensor("psib_bc", [128, H], F32, kind="ExternalInput")
    d_psibT = nc.dram_tensor("psibT", [128, KT_H], F32, kind="ExternalInput")
    d_phibT4 = nc.dram_tensor("phibT4", [128, KT_H], F16, kind="ExternalInput")
    d_sel8f8 = nc.dram_tensor("sel8f8", [8, 2, 16], E4, kind="ExternalInput")
    d_one2 = nc.dram_tensor("one2", [2, 2, 16], E4, kind="ExternalInput")
    d_b1half = nc.dram_tensor("b1half", [2, 2, G], E4, kind="ExternalInput")
    d_emb8 = nc.dram_tensor("emb8", [T_steps, 8, 2, G], E4, kind="ExternalInput")
    d_owT = nc.dram_tensor("owT", [KC, V], F16, kind="ExternalInput")
    d_out = nc.dram_tensor("out", [T_steps * BS, V], F16, kind="ExternalOutput")

    NTB = T_steps * BS
    NBLK = NTB // 128  # complete 128-row output blocks

    with tile.TileContext(nc) as tc:
        with (
            tc.tile_pool(name="persist", bufs=1) as persist,
            tc.tile_pool(name="work", bufs=2) as work,
        ):
            identF = persist.tile([128, 128], F16)
            make_identity(nc, identF)

            # persistent state (items on partitions 0..15, 8 real)
            CS0 = persist.tile([128, KT_H, BS], F32)
            CS1 = persist.tile([128, KT_H, BS], F32)
            s0T8 = persist.tile([128, KT_H, 16], E4)
            s1T8 = persist.tile([128, KT_H, 16], E4)
            cT8 = persist.tile([128, KT_H, 16], E4)
            alT0 = persist.tile([128, RT, 128], F16)
            alT1 = persist.tile([128, RT, 128], F16)
            # history [p, k(4 s1 + 4 c), t*BS+b] fp16, loop-resident
            sb_hist = persist.tile([128, KT_KC, NTB], F16)
            gact0 = persist.tile([16, G], F16)
            gact1 = persist.tile([16, G], F16)
            for st in (CS0, CS1):
                nc.vector.memset(st, 0.0)
            for st in (s0T8, s1T8, cT8):
                nc.vector.memset(st, 0.0)

            sb_sel8f8 = persist.tile([8, 2, 16], E4)
            nc.sync.dma_start(sb_sel8f8, d_sel8f8[:])
            sb_one2 = persist.tile([2, 2, 16], E4)
            nc.sync.dma_start(sb_one2, d_one2[:])
            sb_b1half = persist.tile([2, 2, G], E4)
            nc.sync.dma_start(sb_b1half, d_b1half[:])
            sb_psibT = persist.tile([128, KT_H], F32)
            nc.sync.dma_start(sb_psibT, d_psibT[:])
            sb_phibT4 = persist.tile([128, KT_H], F16)
            nc.sync.dma_start(sb_phibT4, d_phibT4[:])
            ring = [
                persist.tile([8, 2, G], E4, name=f"ring{i}", tag=f"ring{i}")
                for i in range(3)
            ]

            with tc.tile_pool(name="wts", bufs=1) as wts:
                sb_W0 = wts.tile([128, 4, 2, G], E4)
                nc.sync.dma_start(sb_W0, d_W0[:])
                sb_W1 = wts.tile([128, 4, 2, G], E4)
                nc.sync.dma_start(sb_W1, d_W1[:])
                sb_hp = wts.tile([128, RT * BS, H], F16)  # [p, rt*BS+b, d]
                sb_hqT = wts.tile([128, KT_H * BS, R], F16)  # phi^T hp, [p, dt*BS+b, r]
                sb_EbT = wts.tile([128, RT * BS], F16)  # exp(phib.hp*scale), [r, rt*BS+b]

                # ---------------- Phase 1 ----------------
                with (
                    tc.tile_pool(name="ph1", bufs=1) as ph1,
                    tc.tile_pool(name="pp1", bufs=2, space="PSUM") as pp1,
                ):
                    NBR = BS * R  # 2048
                    sb_hT = ph1.tile([128, KT_H, NBR], F16)
                    nc.sync.dma_start(sb_hT, d_hT.rearrange("(kt p) n -> p kt n", p=128))
                    sb_psiT = ph1.tile([128, KT_H, H], F16)
                    nc.sync.dma_start(
                        sb_psiT, d_psiT.rearrange("(kt p) f -> p kt f", p=128)
                    )
                    sb_psib = ph1.tile([128, H], F32)
                    nc.sync.dma_start(sb_psib, d_psib_bc[:])
                    sb_phiT = ph1.tile([128, KT_H, H], F16)
                    nc.sync.dma_start(
                        sb_phiT, d_phiT.rearrange("(kt p) f -> p kt f", p=128)
                    )
                    sb_hpT = ph1.tile([128, KT_H * BS, R], F16)  # [p, dt*BS+b, r]

                    # hp (r-on-partition): act-stationary GEMM
                    for m in range(NBR // 128):  # 16
                        ps = pp1.tile([128, H], F32, tag="pp1", name="ps1")
                        for kt in range(KT_H):
                            nc.tensor.matmul(
                                ps,
                                lhsT=sb_hT[:, kt, m * 128 : (m + 1) * 128],
                                rhs=sb_psiT[:, kt, :],
                                start=(kt == 0),
                                stop=(kt == KT_H - 1),
                            )
                        b_, rt_ = divmod(m, RT)
                        nc.vector.tensor_add(sb_hp[:, rt_ * BS + b_, :], ps, sb_psib)
                    # hpT (d-on-partition): weight-stationary GEMM
                    for mt in range(KT_H):
                        for nch in range(NBR // 512):  # 4
                            ps = pp1.tile([128, H], F32, tag="pp1", name="ps2")
                            for kt in range(KT_H):
                                nc.tensor.matmul(
                                    ps,
                                    lhsT=sb_psiT[:, kt, mt * 128 : (mt + 1) * 128],
                                    rhs=sb_hT[:, kt, nch * 512 : (nch + 1) * 512],
                                    start=(kt == 0),
                                    stop=(kt == KT_H - 1),
                                )
                            for j in range(512 // R):  # 2 items per chunk
                                b_ = nch * 2 + j
                                nc.vector.tensor_scalar_add(
                                    sb_hpT[:, mt * BS + b_, :],
                                    ps[:, j * R : (j + 1) * R],
                                    sb_psibT[:, mt : mt + 1],
                                )

                    # e_base[r,(rt,b)] = phib . hp[b, rt*128+r, :] * scale
                    pEb = pp1.tile([128, RT * BS], F32, tag="eb", name="pEb")
                    for b_ in range(BS):
                        for rt_ in range(RT):
                            for dt in range(KT_H):
                                nc.tensor.matmul(
                                    pEb[:, rt_ * BS + b_ : rt_ * BS + b_ + 1],
                                    lhsT=sb_hpT[
                                        :, dt * BS + b_, rt_ * 128 : (rt_ + 1) * 128
                                    ],
                                    rhs=sb_phibT4[:, dt : dt + 1],
                                    start=(dt == 0),
                                    stop=(dt == KT_H - 1),
                                )
                    sb_Eb32 = ph1.tile([128, RT * BS], F32)
                    nc.scalar.activation(sb_Eb32, pEb, AF.Exp)
                    nc.vector.tensor_copy(sb_EbT, sb_Eb32)
                    # hq^T = (phi*scale*0.5)^T hp  (for scores vs 2*s1)
                    for mt in range(KT_H):
                        for b_ in range(BS):
                            ps = pp1.tile([128, R], F32, tag="pp1", name="ps3")
                            for dt in range(KT_H):
                                nc.tensor.matmul(
                                    ps,
                                    lhsT=sb_phiT[:, dt, mt * 128 : (mt + 1) * 128],
                                    rhs=sb_hpT[:, dt * BS + b_, :],
                                    start=(dt == 0),
                                    stop=(dt == KT_H - 1),
                                )
                            nc.vector.tensor_copy(sb_hqT[:, mt * BS + b_, :], ps)
                    # prescale hp rows by E_base (context absorbs the e_base
                    # softmax weighting; denominator comes from EbT matmuls)
                    for rt_ in range(RT):
                        for b_ in range(BS):
                            nc.vector.tensor_scalar_mul(
                                sb_hp[:, rt_ * BS + b_, :],
                                sb_hp[:, rt_ * BS + b_, :],
                                sb_Eb32[:, rt_ * BS + b_ : rt_ * BS + b_ + 1],
                            )

                # output-projection weights, resident (own pool: reuses the
                # SBUF freed by ph1 -- wts would reserve it for the whole run)
                _owt_cm = tc.tile_pool(name="owt", bufs=1)
                owt = _owt_cm.__enter__()
                sb_owT = owt.tile([128, KT_KC, V], F16)
                nc.sync.dma_start(
                    sb_owT, d_owT.rearrange("(k p) v -> p k v", p=128)
                )

                _p2cms = [
                    tc.tile_pool(name="pgate", bufs=4, space="PSUM"),
                    tc.tile_pool(name="pc", bufs=2, space="PSUM"),
                    tc.tile_pool(name="psmall", bufs=1, space="PSUM"),
                    tc.tile_pool(name="psp", bufs=1, space="PSUM"),
                ]
                pgate, pcp, psmall, pspp = [cm.__enter__() for cm in _p2cms]

                for tpre in range(min(2, T_steps)):
                    nc.sync.dma_start(ring[tpre], d_emb8[tpre])

                open_ps = {}  # (layer, ch) -> psum tile with open accum group

                def layer_open(li, W, bias_lhsT, bias_rhs, second8):
                    """Open all 4 chunk psum groups: bias/emb MM + prev-state
                    MMs (skt 2,3).  All operands ready before this emits."""
                    for ch in CH_ORDER:
                        csl = slice(ch * 512, (ch + 1) * 512)
                        ps = pgate.tile([16, 512], F32, tag="pg", name=f"pg{li}{ch}")
                        nc.tensor.matmul(
                            ps, lhsT=bias_lhsT, rhs=bias_rhs[:, :, csl],
                            start=True, stop=False, perf_mode=DR,
                        )
                        # close this group (stop=True): the late MMs form a
                        # second start=False group accumulating on top, so
                        # the scheduler can place the two halves separately
                        for skt in (2, 3):
                            nc.tensor.matmul(
                                ps,
                                lhsT=second8[:, 2 * (skt % 2) : 2 * (skt % 2) + 2, :],
                                rhs=W[:, skt, :, csl],
                                start=False, stop=(skt == 3), perf_mode=DR,
                            )
                        open_ps[(li, ch)] = ps

                def layer_close(li, W, first8, gact):
                    """Finish chunks (skt 0,1 on the fresh operand), activate,
                    transpose gates into [128, gate, kt, 8] and start the
                    cell chain (through CS update) in transposed space."""
                    gT = psmall.tile(
                        [128, 4, KT_H, BS], F16, tag="sh", name=f"gT{li}"
                    )
                    for ch in CH_ORDER:
                        csl = slice(ch * 512, (ch + 1) * 512)
                        ps = open_ps.pop((li, ch))
                        for skt in (0, 1):
                            nc.tensor.matmul(
                                ps,
                                lhsT=first8[:, 2 * (skt % 2) : 2 * (skt % 2) + 2, :],
                                rhs=W[:, skt, :, csl],
                                start=False, stop=(skt == 1), perf_mode=DR,
                            )
                        nc.scalar.activation(
                            gact[:, csl], ps, AF.Tanh, scale=1.0 / S
                        )
                    return gT

                def transp_gate(gT, gact, ch):
                    for kt in range(KT_H):
                        nc.tensor.transpose(
                            gT[:, ch, kt, :],
                            gact[0:BS, ch * 512 + kt * 128 : ch * 512 + (kt + 1) * 128],
                            identF[0:BS, 0:BS],
                        )

                def cell_mid(gT, gact, CS):
                    """Transposes for g,i,f + Bv/Av/CS updates (all [128,32])."""
                    for ch in (3, 0, 1):
                        transp_gate(gT, gact, ch)
                    ti, tf = gT[:, 0], gT[:, 1]
                    tgS = work.tile([128, KT_H, BS], F16, tag="tgS")
                    nc.vector.tensor_copy(tgS, gT[:, 3])  # one PSUM read/op max
                    Bv = work.tile([128, KT_H, BS], F16, tag="Bv")
                    Av = work.tile([128, KT_H, BS], F32, tag="Av")
                    nc.vector.scalar_tensor_tensor(Bv, ti, 1.0, tgS, ALU.add, ALU.mult)
                    nc.vector.scalar_tensor_tensor(Av, tf, 1.0, CS, ALU.add, ALU.mult)
                    nc.vector.scalar_tensor_tensor(CS, Av, 0.5, Bv, ALU.mult, ALU.add)

                def cell_finish(gT, gact, CS, s_fp8, hist_slice):
                    """o-transpose + s' = (to+1)*tanh(CS/2), written straight
                    into the fp8 stationary (and f16 history for s1)."""
                    transp_gate(gT, gact, 2)
                    to = gT[:, 2]
                    tch = work.tile([128, KT_H, BS], F16, tag="tch")
                    nc.scalar.activation(tch, CS, AF.Tanh, scale=0.5)
                    if hist_slice is not None:
                        nc.vector.scalar_tensor_tensor(
                            hist_slice, to, 1.0, tch, ALU.add, ALU.mult
                        )
                        nc.vector.tensor_copy(s_fp8[:, :, :BS], hist_slice)
                    else:
                        nc.vector.scalar_tensor_tensor(
                            s_fp8[:, :, :BS], to, 1.0, tch, ALU.add, ALU.mult
                        )

                # ---- output projection bookkeeping ----
                proj_chunks = [(m, nch) for m in range(NBLK) for nch in range(8)]
                proj_next = [0]  # next chunk index to emit in-loop

                def emit_proj(t):
                    """One [128,512] projection chunk if its block is ready."""
                    if proj_next[0] >= len(proj_chunks):
                        return
                    m, nch = proj_chunks[proj_next[0]]
                    if t < 16 * (m + 1) + 1:
                        return
                    proj_next[0] += 1
                    ps = pspp.tile([128, 512], F32, tag="pe", name="proj")
                    for kt in range(KT_KC):
                        nc.tensor.matmul(
                            ps,
                            lhsT=sb_hist[:, kt, m * 128 : (m + 1) * 128],
                            rhs=sb_owT[:, kt, nch * 512 : (nch + 1) * 512],
                            start=(kt == 0),
                            stop=(kt == KT_KC - 1),
                        )
                    ost = work.tile([128, 512], F16, tag="ost", name="ost")
                    nc.vector.tensor_copy(ost, ps)
                    nc.sync.dma_start(
                        d_out[m * 128 : (m + 1) * 128, nch * 512 : (nch + 1) * 512],
                        ost,
                    )

                def attention(t):
                    """score -> exp -> context (+1/sum via EbT MMs) -> cT8;
                    also opens L0(t+1)'s gate psum groups."""
                    if t >= 0:
                        pe = pspp.tile([128, 2, R], F32, tag="pe", name="pe")
                        for b_ in range(BS):
                            g = b_ // 4
                            j = (b_ % 4) * 32
                            col = t * BS + b_
                            for dt in range(KT_H):
                                nc.tensor.matmul(
                                    pe[j : j + 1, g, :],
                                    lhsT=hist_s1[:, 0, dt, col : col + 1],
                                    rhs=sb_hqT[:, dt * BS + b_, :],
                                    start=(dt == 0),
                                    stop=(dt == KT_H - 1),
                                    tile_position=(0, j),
                                )
                    # L0(t+1) opens stream while softmax runs on Act/DVE
                    if 0 <= t + 1 < T_steps:
                        layer_open(0, sb_W0, sb_sel8f8, ring[(t + 1) % 3], s0T8)

                    if t >= 0:
                        # single exp over both groups; no max-subtract (|e|<<1)
                        al = work.tile([128, 2, R], F16, tag="al", name="al")
                        nc.scalar.activation(al, pe, AF.Exp)
                        pta = pspp.tile([128, 2, RT, 128], F16, tag="pe", name="pta")
                        for gi, alT in enumerate((alT0, alT1)):
                            for rt_ in range(RT):
                                nc.tensor.transpose(
                                    pta[:, gi, rt_, :],
                                    al[:, gi, rt_ * 128 : (rt_ + 1) * 128], identF,
                                )
                            nc.vector.tensor_copy(alT, pta[:, gi])
                    # context (unnorm., hp prescaled by E_base) + denominator
                    pc0 = pcp.tile([128, H], F32, tag="pc", name="pc0")
                    pc1 = pcp.tile([128, H], F32, tag="pc", name="pc1")
                    pdn = pspp.tile([128, 2], F32, tag="pe", name="pdn")
                    for b_ in range(BS):
                        ps = pc0 if b_ < 4 else pc1
                        g = b_ // 4
                        j = (b_ % 4) * 32
                        alTx = alT0 if b_ < 4 else alT1
                        for rt_ in range(RT):
                            nc.tensor.matmul(
                                ps[j : j + 1, :],
                                lhsT=alTx[:, rt_, j : j + 1],
                                rhs=sb_hp[:, rt_ * BS + b_, :],
                                start=(rt_ == 0),
                                stop=(rt_ == RT - 1),
                                tile_position=(0, j),
                            )
                        for rt_ in range(RT):
                            nc.tensor.matmul(
                                pdn[j : j + 1, g : g + 1],
                                lhsT=alTx[:, rt_, j : j + 1],
                                rhs=sb_EbT[:, rt_ * BS + b_ : rt_ * BS + b_ + 1],
                                start=(rt_ == 0),
                                stop=(rt_ == RT - 1),
                                tile_position=(0, j),
                            )
                    rc = work.tile([128, 2], F32, tag="rc")
                    nc.vector.reciprocal(rc, pdn)
                    cstr0 = work.tile([128, H], F16, tag="cstr0")
                    cstr1 = work.tile([128, H], F16, tag="cstr1")
                    nc.vector.tensor_scalar_mul(cstr0, pc0, rc[:, 0:1])
                    nc.vector.tensor_scalar_mul(cstr1, pc1, rc[:, 1:2])
                    # transpose: item at col 32j -> cT8 (+hist c) compact cols
                    for gi, csx in ((0, cstr0), (1, cstr1)):
                        ptc = psmall.tile([128, KT_H, 128], F16, tag="sh", name="ptc")
                        for chk in range(KT_H):
                            nc.tensor.transpose(
                                ptc[:, chk, :],
                                csx[:, chk * 128 : (chk + 1) * 128],
                                identF,
                            )
                        src = ptc.rearrange("p k (i s) -> p k i s", s=32)[:, :, :, 0]
                        nc.vector.tensor_copy(cT8[:, :, gi * 4 : gi * 4 + 4], src)
                        if t >= 0:
                            nc.vector.tensor_copy(
                                sb_hist.rearrange("p (g k) n -> p g k n", g=2)[
                                    :, 1, :, t * BS + gi * 4 : t * BS + gi * 4 + 4
                                ],
                                src,
                            )

                # ---------------- Phase 2 ----------------
                hist_s1 = sb_hist.rearrange("p (g k) n -> p g k n", g=2)

                nc.vector.memset(alT0, 1.0)
                nc.vector.memset(alT1, 1.0)
                attention(-1)  # c_init: alpha prop. to E_base; opens L0(0)

                for t in range(T_steps):
                    gT0 = layer_close(0, sb_W0, cT8, gact0)
                    cell_mid(gT0, gact0, CS0)
                    layer_open(1, sb_W1, sb_one2, sb_b1half, s1T8)
                    if t % 2 == 0:
                        emit_proj(t)  # fill L0 tail
                    cell_finish(gT0, gact0, CS0, s0T8, None)

                    gT1 = layer_close(1, sb_W1, s0T8, gact1)
                    cell_mid(gT1, gact1, CS1)
                    if t % 2 == 1:
                        emit_proj(t)  # fill L1 tail
                    cell_finish(
                        gT1, gact1, CS1, s1T8,
                        hist_s1[:, 0, :, t * BS : t * BS + BS],
                    )

                    attention(t)

                    if t + 2 < T_steps:
                        nc.sync.dma_start(ring[(t + 2) % 3], d_emb8[t + 2])

                for cm in reversed(_p2cms):
                    cm.__exit__(None, None, None)

                # ---------------- Phase 3: remaining output projection ------
                with (
                    tc.tile_pool(name="ph3w", bufs=2) as ph3w,
                    tc.tile_pool(name="pp3", bufs=4, space="PSUM") as pp3,
                ):
                    rem = list(proj_chunks[proj_next[0] :])
                    if NTB - NBLK * 128:  # partial block (T_steps < 16)
                        rem += [(NBLK, nch) for nch in range(8)]
                    for m, nch in rem:
                        rows = min(128, NTB - m * 128)
                        ps = pp3.tile([128, 512], F32, tag="po", name="po")
                        for kt in range(KT_KC):
                            nc.tensor.matmul(
                                ps[:rows, :],
                                lhsT=sb_hist[:, kt, m * 128 : m * 128 + rows],
                                rhs=sb_owT[:, kt, nch * 512 : (nch + 1) * 512],
                                start=(kt == 0),
                                stop=(kt == KT_KC - 1),
                            )
                        ost = ph3w.tile([128, 512], F16, tag="ost", name="ost3")
                        nc.vector.tensor_copy(ost[:rows, :], ps[:rows, :])
                        nc.sync.dma_start(
                            d_out[m * 128 : m * 128 + rows,
                                  nch * 512 : (nch + 1) * 512],
                            ost[:rows, :],
                        )
                _owt_cm.__exit__(None, None, None)
    nc.compile()
    return nc


def host_prep(inputs, T_steps=T):
    f = lambda k: np.asarray(inputs[k], np.float32)
    h = f("h")
    y = np.asarray(inputs["y"])
    scale = 1.0 / math.sqrt(H)
    # gate reorder i,f,g,o -> i,f,o,g; i/f/o rows x0.5 (tanh-form sigmoid)
    perm = np.concatenate(
        [np.arange(H), H + np.arange(H), 3 * H + np.arange(H), 2 * H + np.arange(H)]
    )
    gs = np.concatenate([np.full(3 * H, 0.5), np.ones(H)]).astype(np.float32)[:, None]
    w_ih0, w_hh0 = f("w_ih0")[perm], f("w_hh0")[perm]
    w_ih1, w_hh1 = f("w_ih1")[perm], f("w_hh1")[perm]
    b0 = (f("b_ih0") + f("b_hh0"))[perm]
    b1 = (f("b_ih1") + f("b_hh1"))[perm]
    # state inputs are stored as 2*s -> their weight columns x0.5
    W0 = np.concatenate([w_ih0[:, V:], w_hh0 * 0.5], axis=1) * gs  # [G, KC]
    W1 = np.concatenate([w_ih1 * 0.5, w_hh1 * 0.5], axis=1) * gs

    def pack8(Wm):  # [G, KC] -> [128, skt 4, plane 2, G] fp8 (scaled by S)
        Wt = np.ascontiguousarray(Wm.T * S)  # [KC, G]
        return np.ascontiguousarray(
            Wt.reshape(4, 2, 128, G).transpose(2, 0, 1, 3)
        ).astype(ml_dtypes.float8_e4m3)

    fp8 = lambda x: np.ascontiguousarray(x).astype(ml_dtypes.float8_e4m3)

    embW = w_ih0[:, :V] * gs
    # item b at (b%4, b//4); rows 4-7 carry the fp8 quantization residual
    sel8 = np.zeros((8, 2, 16), np.float32)
    for b_ in range(BS):
        sel8[b_ % 4, b_ // 4, b_] = 1.0
        sel8[4 + b_ % 4, b_ // 4, b_] = 1.0
    one2 = np.ones((2, 2, 16), np.float32)
    b1h = (b1 * gs[:, 0] * S * 0.5)[None, None, :]
    b1q = fp8(np.broadcast_to(b1h, (1, 2, G)))
    b1r = fp8(np.broadcast_to(b1h, (1, 2, G)) - b1q.astype(np.float32))
    b1half = np.concatenate([b1q.astype(np.float32), b1r.astype(np.float32)], axis=0)

    phiT = (f("phi_w") * scale * 0.5).T.astype(np.float16)  # [h_in, d_out]
    psiT = f("psi_w").T
    psi_b = f("psi_b")
    psibT = np.ascontiguousarray(psi_b.reshape(KT_H, 128).T)
    psib_bc = np.ascontiguousarray(np.tile(psi_b[None, :], (128, 1)))
    phibT4 = np.ascontiguousarray(
        (f("phi_b") * scale).reshape(KT_H, 128).T
    ).astype(np.float16)
    oW = f("out_w").copy()
    oW[:, :H] *= 0.5  # s1 history stored as 2*s1
    owT = np.ascontiguousarray(oW.T)
    c16 = lambda x: np.ascontiguousarray(x.astype(np.float16))
    shared = dict(
        W0=pack8(W0), W1=pack8(W1), phiT=c16(phiT),
        psiT=c16(psiT), psib_bc=psib_bc, psibT=psibT, phibT4=phibT4,
        sel8f8=fp8(sel8), one2=fp8(one2), b1half=fp8(b1half),
        owT=c16(owT),
    )
    in_maps = []
    for ci in range(NCORES):
        sl = slice(ci * BS, (ci + 1) * BS)
        m = dict(shared)
        m["hT"] = c16(h[sl].reshape(BS * R, H).T)
        # emb rows for this core's y slice: [T, 8(p), 2(pl), G]; p<4 holds the
        # fp8 value for item b=pl*4+p, p>=4 the quantization residual
        emb_c = (embW.T[y[sl, :T_steps]] + (b0 * gs[:, 0])[None, None, :]) * S
        e_m = emb_c.transpose(1, 0, 2).reshape(T_steps, 2, 4, G).transpose(0, 2, 1, 3)
        e_q = fp8(e_m)
        e_r = fp8(e_m - e_q.astype(np.float32))
        m["emb8"] = np.ascontiguousarray(
            np.concatenate([e_q, e_r], axis=1)
        )
        in_maps.append(m)
    return in_maps


def gather_output(per_core_outs, out_b, T_steps=T):
    """per-core [T*8, V] f16 -> [B, T, V] f32 (+ out_b, applied host-side)."""
    shards = []
    for o in per_core_outs:
        o = np.asarray(o, np.float32).reshape(T_steps, BS, V)
        shards.append(np.ascontiguousarray(o.transpose(1, 0, 2)))
    return np.concatenate(shards, axis=0) + np.asarray(
        out_b, np.float32
    )[None, None, :]


def kernel(**inputs):
    nc = build_program(T)
    in_maps = host_prep(inputs, T)
    res = run_bass_kernel_spmd(nc, in_maps, list(range(NCORES)))
    return gather_output(
        [res.results[ci]["out"] for ci in range(NCORES)], inputs["out_b"]
    )
